# revision 14
# baseline (speedup 1.0000x reference)
"""Trainium2 Bass kernel for a 3x3 'same' conv: x [8,16,512,512] f32, weight [16,144].

Strategy (data-parallel over batch, 1 image per NeuronCore, 8 cores):
  - Host pads x columns: xp [16, 512, 514] with zero cols 0 and 513, so the
    horizontal taps are plain SBUF column offsets and no on-device memsets or
    edge fixups are needed.
  - Row-group the image into 64 groups of R=8 output rows. A group loads a
    10-row window [8g-1, 8g+9) clamped to the image; SBUF x-tile partition
    p = ci*5 + jj holds TWO consecutive padded rows (Y+2jj, Y+2jj+1) in its
    free dim -> 4112-byte DMA packets (2x fewer, larger packets than a
    row-per-partition layout; DMA engine packet throughput is the bottleneck).
  - K = 80 contraction partitions (16 ci x 5 row-pair slots), M = 128 psum
    partitions (16 co x 8 rows). Six accumulating matmuls per group
    (3 kw taps x 2 row parities) into one full PSUM bank [128, 512].
    Stationary variants (kw, parity h, boundary b) are built on the host from
    the [16,144] weight: wk[ci*5+jj, co*8+r] = w[co,ci,kh,kw] where
    2jj + h = r + kh + (b-1), entries falling outside the window dropped
    (those are the zero-pad rows).
  - PSUM -> SBUF via VectorE copy [128, 512], then DMA to HBM (all 16 DMA
    ports balanced). Input DMAs issue on the sync queue, output on scalar.
  - float32r matmul dtype: full-rate fp32 on the PE (4x faster than float32).
"""

import os
from contextlib import ExitStack

import numpy as np

C_OUT, C_IN, KH, KW = 16, 16, 3, 3
H = W = 512
WP = W + 2      # host-padded row length
B = 8
R = 8           # output rows per group
S = 5           # row-pair slots per group (10 input rows)
M = C_OUT * R   # 128 psum partitions
K = C_IN * S    # 80 contraction partitions
NV = KW * 2 * 3  # stationary variants: kw x parity x boundary
G = H // R      # 64 groups

# matmul dtype: "float32r" (full-rate, relaxed precision) or "float32" (exact, 1/4 rate)
MM_DTYPE_NAME = os.environ.get("CONV_MM_DTYPE", "float32r")

_CACHE = {}


def _build_weights(weight: np.ndarray) -> np.ndarray:
    """[16,144] -> [80, 18*128] stationary matrices, variant v = (kw*2+h)*3+b.

    wk[ci*S+jj, v, co*R+r] = w[co, ci, kh, kw] where 2*jj + h = r + kh + (b-1);
    (r, kh) pairs whose target row falls outside the 10-row window are dropped
    (those reference the zero-pad rows above/below the image).
    """
    w = np.asarray(weight, dtype=np.float32).reshape(C_OUT, C_IN, KH, KW)
    wk = np.zeros((K, NV, M), np.float32)
    for kw in range(KW):
        for h in range(2):
            for b in range(3):
                v = (kw * 2 + h) * 3 + b
                d = b - 1
                for co in range(C_OUT):
                    for r in range(R):
                        for kh in range(KH):
                            t = r + kh + d  # = 2*jj + h
                            if t % 2 == h and 0 <= t <= 2 * S - 1:
                                jj = t // 2
                                for ci in range(C_IN):
                                    wk[ci * S + jj, v, co * R + r] = w[co, ci, kh, kw]
    return np.ascontiguousarray(wk.reshape(K, NV * M))


def _build_nc():
    import concourse.tile as tile
    from concourse import bacc, mybir

    mm_dt = getattr(mybir.dt, MM_DTYPE_NAME)
    f32 = mybir.dt.float32

    nc = bacc.Bacc(
        "TRN2", target_bir_lowering=False, debug=False,
        enable_asserts=False, num_devices=B,
    )
    # float32r has the same 4-byte layout as float32; declaring the inputs as
    # the matmul dtype keeps the BIR fp32r producer->consumer chain consistent.
    x = nc.dram_tensor("x", [C_IN, H, WP], mm_dt, kind="ExternalInput").ap()
    wkin = nc.dram_tensor("wk", [K, NV * M], mm_dt, kind="ExternalInput").ap()
    out = nc.dram_tensor("out", [C_OUT, H, W], f32, kind="ExternalOutput").ap()

    with tile.TileContext(nc) as tc, ExitStack() as ctx:
        wpool = ctx.enter_context(tc.tile_pool(name="wpool", bufs=1))
        xpool = ctx.enter_context(tc.tile_pool(name="xpool", bufs=8))
        opool = ctx.enter_context(tc.tile_pool(name="opool", bufs=8))
        ppool = ctx.enter_context(tc.tile_pool(name="ppool", bufs=8, space="PSUM"))

        wt = wpool.tile([K, NV * M], mm_dt, name="wt")
        nc.scalar.dma_start(out=wt[:], in_=wkin[:])

        for g in range(G):
            b = 0 if g == 0 else (2 if g == G - 1 else 1)
            Y = min(max(R * g - 1, 0), H - 2 * S)

            xtile = xpool.tile([K, 2 * WP], mm_dt, name="xtile", tag="xtile")
            nc.sync.dma_start(out=xtile[:], in_=x[:, Y : Y + 2 * S, :])

            pt = ppool.tile([M, W], f32, name="pt", tag="pt")
            first = True
            for kw in range(KW):
                for h in range(2):
                    v = (kw * 2 + h) * 3 + b
                    nc.tensor.matmul(
                        pt[:, 0:W],
                        wt[:, v * M : (v + 1) * M],
                        xtile[:, h * WP + kw : h * WP + kw + W],
                        start=first,
                        stop=(kw == KW - 1 and h == 1),
                    )
                    first = False

            ot = opool.tile([M, W], f32, name="ot", tag="ot")
            nc.vector.tensor_copy(ot[:], pt[:])
            nc.scalar.dma_start(out=out[:, R * g : R * g + R, :], in_=ot[:])

    nc.compile()
    return nc


def get_nc():
    if "nc" not in _CACHE:
        _CACHE["nc"] = _build_nc()
    return _CACHE["nc"]


def run(x: np.ndarray, weight: np.ndarray, **spmd_kwargs):
    """Run the conv on 8 cores; returns (out [8,16,512,512], BassKernelResults)."""
    from concourse.bass_utils import run_bass_kernel_spmd

    x = np.asarray(x, dtype=np.float32)
    xp = np.zeros((B, C_IN, H, WP), np.float32)
    xp[:, :, :, 1 : W + 1] = x
    wk = _build_weights(weight)
    nc = get_nc()
    in_maps = [{"x": xp[b], "wk": wk} for b in range(B)]
    res = run_bass_kernel_spmd(nc, in_maps, list(range(B)), **spmd_kwargs)
    out = np.stack([res.results[b]["out"] for b in range(B)], axis=0)
    return out, res


def kernel(x: np.ndarray, weight: np.ndarray) -> np.ndarray:
    return run(x, weight)[0]


# revision 17
# speedup vs baseline: 1.0180x; 1.0180x over previous
"""Trainium2 Bass kernel for a 3x3 'same' conv: x [8,16,512,512] f32, weight [16,144].

Data-parallel over batch: 1 image per NeuronCore, 8 cores. The kernel is
DMA-byte-bound (~225 GB/s effective per core for mixed HBM read+write), so the
shipping mode moves x as fp16 (half the input bytes) while accumulating in
fp32 PSUM.

Modes (CONV_MODE env):
  f16  (default): x/weights fp16, fp32 PSUM + fp32 output.
        R=8 output rows per group, x-tile packs TWO consecutive padded rows
        per partition (p = ci*5 + jj holds rows Y+2jj, Y+2jj+1) -> K = 80,
        M = 128 (16 co x 8 rows), 6 accumulating matmuls per group
        (3 kw taps x 2 row parities). fp16 matmul streams at the full
        2.4 GHz clock, so the extra passes are affordable.
  f16o: like f16 but the output is written as fp16 too (host upcasts);
        fewest bytes, adds ~2.4e-4 output rounding error.
  f32r: all-fp32 path (relaxed-precision fp32r matmul, which streams at
        1.2 GHz -> pass count matters): R=6, row-per-partition K=128, M=96,
        3 matmuls per group. Most accurate (~1.5e-4), most bytes.

Common structure:
  - Host pads x columns to [16, 512, 514] with zero cols 0 and 513 so the
    horizontal taps become plain SBUF column offsets (no device memsets,
    full-bank PSUM writes -- the fp32r dst-pattern ISA restriction).
  - Stationary weight variants per (kw tap, row parity, boundary) are built
    on the host from the [16,144] weight; entries whose target row falls
    outside the loaded window are dropped (those are the zero-pad rows).
  - PSUM -> SBUF via VectorE copy, input DMAs on the sync HWDGE queue,
    output DMAs on the scalar HWDGE queue.
"""

import os

import numpy as np

C_OUT, C_IN, KH, KW = 16, 16, 3, 3
H = W = 512
WP = W + 2      # host-padded row length
B = 8

MODE = os.environ.get("CONV_MODE", "f16")  # f16 | f16o | f32r

_CACHE = {}


# ---------------------------------------------------------------- f32r mode
R6, J6 = 6, 8
M6 = C_OUT * R6          # 96
K6 = C_IN * J6           # 128
Y0_6 = [6 * g for g in range(85)] + [506]


def _weights_f32r(weight):
    """[16,144] -> [128, 9*96]; v = kw*3 + b; wk[ci*8+j, v, co*6+r] at
    j = r + kh + (b-1), out-of-window entries dropped."""
    w = np.asarray(weight, dtype=np.float32).reshape(C_OUT, C_IN, KH, KW)
    wk = np.zeros((KW, 3, K6, M6), np.float32)
    for kw in range(KW):
        for b in range(3):
            for co in range(C_OUT):
                for r in range(R6):
                    for kh in range(KH):
                        j = r + kh + (b - 1)
                        if 0 <= j < J6:
                            for ci in range(C_IN):
                                wk[kw, b, ci * J6 + j, co * R6 + r] = w[co, ci, kh, kw]
    return np.ascontiguousarray(wk.transpose(2, 0, 1, 3).reshape(K6, KW * 3 * M6))


def _build_nc_f32r():
    from contextlib import ExitStack

    import concourse.tile as tile
    from concourse import bacc, mybir

    dt_in = mybir.dt.float32r
    f32 = mybir.dt.float32
    NV = KW * 3

    nc = bacc.Bacc("TRN2", target_bir_lowering=False, debug=False,
                   enable_asserts=False, num_devices=B)
    x = nc.dram_tensor("x", [C_IN, H, WP], dt_in, kind="ExternalInput").ap()
    wkin = nc.dram_tensor("wk", [K6, NV * M6], dt_in, kind="ExternalInput").ap()
    out = nc.dram_tensor("out", [C_OUT, H, W], f32, kind="ExternalOutput").ap()

    with tile.TileContext(nc) as tc, ExitStack() as ctx:
        wpool = ctx.enter_context(tc.tile_pool(name="wpool", bufs=1))
        xpool = ctx.enter_context(tc.tile_pool(name="xpool", bufs=8))
        opool = ctx.enter_context(tc.tile_pool(name="opool", bufs=8))
        ppool = ctx.enter_context(tc.tile_pool(name="ppool", bufs=8, space="PSUM"))

        wt = wpool.tile([K6, NV * M6], dt_in, name="wt")
        nc.scalar.dma_start(out=wt[:], in_=wkin[:])

        for y0 in Y0_6:
            b = 0 if y0 == 0 else (2 if y0 == H - R6 else 1)
            Y = min(max(y0 - 1, 0), H - J6)

            xtile = xpool.tile([K6, WP], dt_in, name="xtile", tag="xtile")
            nc.sync.dma_start(out=xtile[:], in_=x[:, Y : Y + J6, :])

            pt = ppool.tile([M6, W], f32, name="pt", tag="pt")
            for kw in range(KW):
                v = kw * 3 + b
                nc.tensor.matmul(pt[:, 0:W], wt[:, v * M6 : (v + 1) * M6],
                                 xtile[:, kw : kw + W],
                                 start=(kw == 0), stop=(kw == KW - 1))

            ot = opool.tile([M6, W], f32, name="ot", tag="ot")
            nc.vector.tensor_copy(ot[:], pt[:])
            nc.scalar.dma_start(out=out[:, y0 : y0 + R6, :], in_=ot[:])

    nc.compile()
    return nc


# ------------------------------------------------------------ f16 / f16o mode
R8, S8 = 8, 5
M8 = C_OUT * R8          # 128
K8 = C_IN * S8           # 80
NV8 = KW * 2 * 3         # kw x parity x boundary
G8 = H // R8             # 64 groups


def _weights_f16(weight):
    """[16,144] -> [80, 18*128] fp16; v = (kw*2+h)*3+b;
    wk[ci*5+jj, v, co*8+r] at 2*jj + h = r + kh + (b-1)."""
    w = np.asarray(weight, dtype=np.float32).reshape(C_OUT, C_IN, KH, KW)
    wk = np.zeros((K8, NV8, M8), np.float32)
    for kw in range(KW):
        for h in range(2):
            for b in range(3):
                v = (kw * 2 + h) * 3 + b
                d = b - 1
                for co in range(C_OUT):
                    for r in range(R8):
                        for kh in range(KH):
                            t = r + kh + d  # = 2*jj + h
                            if t % 2 == h and 0 <= t <= 2 * S8 - 1:
                                jj = t // 2
                                for ci in range(C_IN):
                                    wk[ci * S8 + jj, v, co * R8 + r] = w[co, ci, kh, kw]
    return np.ascontiguousarray(wk.reshape(K8, NV8 * M8).astype(np.float16))


def _build_nc_f16(out_f16: bool):
    from contextlib import ExitStack

    import concourse.tile as tile
    from concourse import bacc, mybir

    f16 = mybir.dt.float16
    f32 = mybir.dt.float32
    dt_out = f16 if out_f16 else f32

    nc = bacc.Bacc("TRN2", target_bir_lowering=False, debug=False,
                   enable_asserts=False, num_devices=B)
    x = nc.dram_tensor("x", [C_IN, H, WP], f16, kind="ExternalInput").ap()
    wkin = nc.dram_tensor("wk", [K8, NV8 * M8], f16, kind="ExternalInput").ap()
    out = nc.dram_tensor("out", [C_OUT, H, W], dt_out, kind="ExternalOutput").ap()

    with tile.TileContext(nc) as tc, ExitStack() as ctx:
        wpool = ctx.enter_context(tc.tile_pool(name="wpool", bufs=1))
        xpool = ctx.enter_context(tc.tile_pool(name="xpool", bufs=8))
        opool = ctx.enter_context(tc.tile_pool(name="opool", bufs=8))
        ppool = ctx.enter_context(tc.tile_pool(name="ppool", bufs=8, space="PSUM"))

        wt = wpool.tile([K8, NV8 * M8], f16, name="wt")
        nc.scalar.dma_start(out=wt[:], in_=wkin[:])

        for g in range(G8):
            b = 0 if g == 0 else (2 if g == G8 - 1 else 1)
            Y = min(max(R8 * g - 1, 0), H - 2 * S8)

            xtile = xpool.tile([K8, 2 * WP], f16, name="xtile", tag="xtile")
            nc.sync.dma_start(out=xtile[:], in_=x[:, Y : Y + 2 * S8, :])

            pt = ppool.tile([M8, W], f32, name="pt", tag="pt")
            first = True
            for kw in range(KW):
                for h in range(2):
                    v = (kw * 2 + h) * 3 + b
                    nc.tensor.matmul(pt[:, 0:W], wt[:, v * M8 : (v + 1) * M8],
                                     xtile[:, h * WP + kw : h * WP + kw + W],
                                     start=first, stop=(kw == KW - 1 and h == 1))
                    first = False

            ot = opool.tile([M8, W], dt_out, name="ot", tag="ot")
            nc.vector.tensor_copy(ot[:], pt[:])
            nc.scalar.dma_start(out=out[:, R8 * g : R8 * g + R8, :], in_=ot[:])

    nc.compile()
    return nc


# ---------------------------------------------------------------------- entry
def get_nc():
    key = MODE
    if key not in _CACHE:
        if MODE == "f32r":
            _CACHE[key] = _build_nc_f32r()
        else:
            _CACHE[key] = _build_nc_f16(out_f16=(MODE == "f16o"))
    return _CACHE[key]


def run(x: np.ndarray, weight: np.ndarray, **spmd_kwargs):
    """Run the conv on 8 cores; returns (out [8,16,512,512] f32, results)."""
    from concourse.bass_utils import run_bass_kernel_spmd

    x = np.asarray(x, dtype=np.float32)
    if MODE == "f32r":
        xp = np.zeros((B, C_IN, H, WP), np.float32)
        xp[:, :, :, 1 : W + 1] = x
        wk = _weights_f32r(weight)
    else:
        xp = np.zeros((B, C_IN, H, WP), np.float16)
        xp[:, :, :, 1 : W + 1] = x.astype(np.float16)
        wk = _weights_f16(weight)

    nc = get_nc()
    in_maps = [{"x": xp[b], "wk": wk} for b in range(B)]
    res = run_bass_kernel_spmd(nc, in_maps, list(range(B)), **spmd_kwargs)
    out = np.stack([res.results[b]["out"] for b in range(B)], axis=0)
    if out.dtype != np.float32:
        out = out.astype(np.float32)
    return out, res


def kernel(x: np.ndarray, weight: np.ndarray) -> np.ndarray:
    return run(x, weight)[0]


# revision 18
# speedup vs baseline: 1.6393x; 1.6103x over previous
"""Trainium2 Bass kernel for a 3x3 'same' conv: x [8,16,512,512] f32, weight [16,144].

Data-parallel over batch: 1 image per NeuronCore, 8 cores.

Measured facts on this machine that drive the design:
  - The PE streams matmul moving-columns at ~1.2 GHz regardless of dtype
    (flat 426 ns issue rate per 512-column matmul, never clocks up), so PE
    time = (#matmuls x 512 cols) x 0.83 ns. The minimum pass count is 3
    (one per kw tap -- each pass has a fixed horizontal shift), with row
    slots for all (r, kh) combos needing 16*(R+2) <= 128 partitions => R=6
    output rows per group, 86 groups, 258 matmuls ~= 110 us.
  - DMA is the other ~equal cost; it scales with bytes moved (~15 GB/s per
    SDMA engine effective). fp16 x halves input bytes at ~3e-4 extra error.

Modes (CONV_MODE env):
  f16  (default): x/weights fp16, fp32 PSUM accumulate + fp32 output.
  f16o: fp16 output too (host upcasts); fewest bytes, adds ~2.4e-4 rounding.
  f32r: all-fp32 (relaxed-precision fp32r matmul); most accurate (~1.5e-4).

Structure (all modes):
  - Host pads x columns to [16, 512, 514] with zero cols 0 and 513 so the
    horizontal taps become plain SBUF column offsets (no device memsets,
    full-bank PSUM writes -- fp32r's dst-pattern ISA restriction).
  - Group g covers output rows [y0, y0+6); its x-tile holds the 8-row padded
    window at partition p = ci*8 + j (row Y+j, Y = clamp(y0-1, 0, 504)),
    K = 128. Three accumulating matmuls (kw = 0,1,2; rhs columns [kw, kw+512))
    into one PSUM bank [96, 512] (M = 16 co x 6 rows).
  - Stationary weights per (kw, boundary variant b): [128, 96] matrices
    wk[ci*8+j, co*6+r] = w[co, ci, j-r-(b-1), kw]; entries whose target row
    falls outside the window are dropped (those are the zero-pad rows).
  - PSUM -> SBUF via VectorE copy; input DMAs on the sync HWDGE queue,
    output DMAs on the scalar HWDGE queue.
"""

import os
from contextlib import ExitStack

import numpy as np

C_OUT, C_IN, KH, KW = 16, 16, 3, 3
H = W = 512
WP = W + 2      # host-padded row length
B = 8
R = 6           # output rows per group
J = R + 2      # input rows per group
M = C_OUT * R   # 96 psum partitions
K = C_IN * J    # 128 contraction partitions
NV = KW * 3     # stationary variants: kw x boundary
GROUP_Y0 = [6 * g for g in range(85)] + [506]

MODE = os.environ.get("CONV_MODE", "f16")  # f16 | f16o | f32r

_CACHE = {}


def _build_weights(weight: np.ndarray) -> np.ndarray:
    """[16,144] -> [128, 9*96] stationary matrices, variant v = kw*3 + b.

    wk[ci*J+j, v, co*R+r] = w[co, ci, kh, kw] at j = r + kh + (b-1); (r, kh)
    with j outside [0, J) dropped (they reference the zero-pad rows).
    """
    w = np.asarray(weight, dtype=np.float32).reshape(C_OUT, C_IN, KH, KW)
    wk = np.zeros((KW, 3, K, M), np.float32)
    for kw in range(KW):
        for b in range(3):
            for co in range(C_OUT):
                for r in range(R):
                    for kh in range(KH):
                        j = r + kh + (b - 1)
                        if 0 <= j < J:
                            for ci in range(C_IN):
                                wk[kw, b, ci * J + j, co * R + r] = w[co, ci, kh, kw]
    out = np.ascontiguousarray(wk.transpose(2, 0, 1, 3).reshape(K, NV * M))
    return out if MODE == "f32r" else out.astype(np.float16)


def _build_nc():
    import concourse.tile as tile
    from concourse import bacc, mybir

    f32 = mybir.dt.float32
    dt_in = mybir.dt.float32r if MODE == "f32r" else mybir.dt.float16
    dt_out = mybir.dt.float16 if MODE == "f16o" else f32

    nc = bacc.Bacc("TRN2", target_bir_lowering=False, debug=False,
                   enable_asserts=False, num_devices=B)
    # for f32r, declaring inputs as the matmul dtype keeps the BIR fp32r
    # producer->consumer chain consistent (same 4-byte layout as float32)
    x = nc.dram_tensor("x", [C_IN, H, WP], dt_in, kind="ExternalInput").ap()
    wkin = nc.dram_tensor("wk", [K, NV * M], dt_in, kind="ExternalInput").ap()
    out = nc.dram_tensor("out", [C_OUT, H, W], dt_out, kind="ExternalOutput").ap()

    with tile.TileContext(nc) as tc, ExitStack() as ctx:
        wpool = ctx.enter_context(tc.tile_pool(name="wpool", bufs=1))
        xpool = ctx.enter_context(tc.tile_pool(name="xpool", bufs=8))
        opool = ctx.enter_context(tc.tile_pool(name="opool", bufs=8))
        ppool = ctx.enter_context(tc.tile_pool(name="ppool", bufs=8, space="PSUM"))

        wt = wpool.tile([K, NV * M], dt_in, name="wt")
        nc.scalar.dma_start(out=wt[:], in_=wkin[:])

        for y0 in GROUP_Y0:
            b = 0 if y0 == 0 else (2 if y0 == H - R else 1)
            Y = min(max(y0 - 1, 0), H - J)

            xtile = xpool.tile([K, WP], dt_in, name="xtile", tag="xtile")
            nc.sync.dma_start(out=xtile[:], in_=x[:, Y : Y + J, :])

            pt = ppool.tile([M, W], f32, name="pt", tag="pt")
            for kw in range(KW):
                v = kw * 3 + b
                nc.tensor.matmul(pt[:, 0:W], wt[:, v * M : (v + 1) * M],
                                 xtile[:, kw : kw + W],
                                 start=(kw == 0), stop=(kw == KW - 1))

            ot = opool.tile([M, W], dt_out, name="ot", tag="ot")
            nc.vector.tensor_copy(ot[:], pt[:])
            nc.scalar.dma_start(out=out[:, y0 : y0 + R, :], in_=ot[:])

    nc.compile()
    return nc


def get_nc():
    if MODE not in _CACHE:
        _CACHE[MODE] = _build_nc()
    return _CACHE[MODE]


def run(x: np.ndarray, weight: np.ndarray, **spmd_kwargs):
    """Run the conv on 8 cores; returns (out [8,16,512,512] f32, results)."""
    from concourse.bass_utils import run_bass_kernel_spmd

    x = np.asarray(x, dtype=np.float32)
    np_in = np.float32 if MODE == "f32r" else np.float16
    xp = np.zeros((B, C_IN, H, WP), np_in)
    xp[:, :, :, 1 : W + 1] = x.astype(np_in)
    wk = _build_weights(weight)
    nc = get_nc()
    in_maps = [{"x": xp[b], "wk": wk} for b in range(B)]
    res = run_bass_kernel_spmd(nc, in_maps, list(range(B)), **spmd_kwargs)
    out = np.stack([res.results[b]["out"] for b in range(B)], axis=0)
    if out.dtype != np.float32:
        out = out.astype(np.float32)
    return out, res


def kernel(x: np.ndarray, weight: np.ndarray) -> np.ndarray:
    return run(x, weight)[0]


# revision 19
# speedup vs baseline: 1.6503x; 1.0067x over previous
"""Trainium2 Bass kernel for a 3x3 'same' conv: x [8,16,512,512] f32, weight [16,144].

Data-parallel over batch: 1 image per NeuronCore, 8 cores.

Measured facts on this machine that drive the design:
  - The PE streams matmul moving-columns at ~1.2 GHz regardless of dtype
    (flat 426 ns issue rate per 512-column matmul, never clocks up), so PE
    time = (#matmuls x 512 cols) x 0.83 ns. The minimum pass count is 3
    (one per kw tap -- each pass has a fixed horizontal shift), with row
    slots for all (r, kh) combos needing 16*(R+2) <= 128 partitions => R=6
    output rows per group, 86 groups, 258 matmuls ~= 110 us.
  - DMA is the other ~equal cost; it scales with bytes moved (~15 GB/s per
    SDMA engine effective). fp16 x halves input bytes at ~3e-4 extra error.

Modes (CONV_MODE env):
  f16  (default): x/weights fp16, fp32 PSUM accumulate + fp32 output.
  f16o: fp16 output too (host upcasts); fewest bytes, adds ~2.4e-4 rounding.
  f32r: all-fp32 (relaxed-precision fp32r matmul); most accurate (~1.5e-4).

Structure (all modes):
  - Host pads x columns to [16, 512, 514] with zero cols 0 and 513 so the
    horizontal taps become plain SBUF column offsets (no device memsets,
    full-bank PSUM writes -- fp32r's dst-pattern ISA restriction).
  - Group g covers output rows [y0, y0+6); its x-tile holds the 8-row padded
    window at partition p = ci*8 + j (row Y+j, Y = clamp(y0-1, 0, 504)),
    K = 128. Three accumulating matmuls (kw = 0,1,2; rhs columns [kw, kw+512))
    into one PSUM bank [96, 512] (M = 16 co x 6 rows).
  - Stationary weights per (kw, boundary variant b): [128, 96] matrices
    wk[ci*8+j, co*6+r] = w[co, ci, j-r-(b-1), kw]; entries whose target row
    falls outside the window are dropped (those are the zero-pad rows).
  - PSUM -> SBUF via VectorE copy; input DMAs on the sync HWDGE queue,
    output DMAs on the scalar HWDGE queue.
"""

import os
from contextlib import ExitStack

import numpy as np

C_OUT, C_IN, KH, KW = 16, 16, 3, 3
H = W = 512
WP = W + 2      # host-padded row length
B = 8
R = 6           # output rows per group
J = R + 2      # input rows per group
M = C_OUT * R   # 96 psum partitions
K = C_IN * J    # 128 contraction partitions
NV = KW * 3     # stationary variants: kw x boundary
GROUP_Y0 = [6 * g for g in range(85)] + [506]

MODE = os.environ.get("CONV_MODE", "f16")  # f16 | f16o | f32r

_CACHE = {}


def _build_weights(weight: np.ndarray) -> np.ndarray:
    """[16,144] -> [128, 9*96] stationary matrices, variant v = kw*3 + b.

    wk[ci*J+j, v, co*R+r] = w[co, ci, kh, kw] at j = r + kh + (b-1); (r, kh)
    with j outside [0, J) dropped (they reference the zero-pad rows).
    """
    w = np.asarray(weight, dtype=np.float32).reshape(C_OUT, C_IN, KH, KW)
    wk = np.zeros((KW, 3, K, M), np.float32)
    for kw in range(KW):
        for b in range(3):
            for co in range(C_OUT):
                for r in range(R):
                    for kh in range(KH):
                        j = r + kh + (b - 1)
                        if 0 <= j < J:
                            for ci in range(C_IN):
                                wk[kw, b, ci * J + j, co * R + r] = w[co, ci, kh, kw]
    out = np.ascontiguousarray(wk.transpose(2, 0, 1, 3).reshape(K, NV * M))
    return out if MODE == "f32r" else out.astype(np.float16)


def _build_nc():
    import concourse.tile as tile
    from concourse import bacc, mybir

    f32 = mybir.dt.float32
    dt_in = mybir.dt.float32r if MODE == "f32r" else mybir.dt.float16
    dt_out = mybir.dt.float16 if MODE == "f16o" else f32

    nc = bacc.Bacc("TRN2", target_bir_lowering=False, debug=False,
                   enable_asserts=False, num_devices=B)
    # for f32r, declaring inputs as the matmul dtype keeps the BIR fp32r
    # producer->consumer chain consistent (same 4-byte layout as float32)
    x = nc.dram_tensor("x", [C_IN, H, WP], dt_in, kind="ExternalInput").ap()
    wkin = nc.dram_tensor("wk", [K, NV * M], dt_in, kind="ExternalInput").ap()
    out = nc.dram_tensor("out", [C_OUT, H, W], dt_out, kind="ExternalOutput").ap()

    with tile.TileContext(nc) as tc, ExitStack() as ctx:
        wpool = ctx.enter_context(tc.tile_pool(name="wpool", bufs=1))
        xpool = ctx.enter_context(tc.tile_pool(name="xpool", bufs=12))
        opool = ctx.enter_context(tc.tile_pool(name="opool", bufs=10))
        ppool = ctx.enter_context(tc.tile_pool(name="ppool", bufs=8, space="PSUM"))

        wt = wpool.tile([K, NV * M], dt_in, name="wt")
        # per-variant weight loads so the first group's stationaries land
        # quickly (first matmul needs only v=0)
        for v in range(NV):
            eng = nc.scalar if v % 2 == 0 else nc.sync
            eng.dma_start(out=wt[:, v * M : (v + 1) * M],
                          in_=wkin[:, v * M : (v + 1) * M])

        for y0 in GROUP_Y0:
            b = 0 if y0 == 0 else (2 if y0 == H - R else 1)
            Y = min(max(y0 - 1, 0), H - J)

            xtile = xpool.tile([K, WP], dt_in, name="xtile", tag="xtile")
            nc.sync.dma_start(out=xtile[:], in_=x[:, Y : Y + J, :])

            pt = ppool.tile([M, W], f32, name="pt", tag="pt")
            for kw in range(KW):
                v = kw * 3 + b
                nc.tensor.matmul(pt[:, 0:W], wt[:, v * M : (v + 1) * M],
                                 xtile[:, kw : kw + W],
                                 start=(kw == 0), stop=(kw == KW - 1))

            ot = opool.tile([M, W], dt_out, name="ot", tag="ot")
            nc.vector.tensor_copy(ot[:], pt[:])
            nc.scalar.dma_start(out=out[:, y0 : y0 + R, :], in_=ot[:])

    nc.compile()
    return nc


def get_nc():
    if MODE not in _CACHE:
        _CACHE[MODE] = _build_nc()
    return _CACHE[MODE]


def run(x: np.ndarray, weight: np.ndarray, **spmd_kwargs):
    """Run the conv on 8 cores; returns (out [8,16,512,512] f32, results)."""
    from concourse.bass_utils import run_bass_kernel_spmd

    x = np.asarray(x, dtype=np.float32)
    np_in = np.float32 if MODE == "f32r" else np.float16
    xp = np.zeros((B, C_IN, H, WP), np_in)
    xp[:, :, :, 1 : W + 1] = x.astype(np_in)
    wk = _build_weights(weight)
    nc = get_nc()
    in_maps = [{"x": xp[b], "wk": wk} for b in range(B)]
    res = run_bass_kernel_spmd(nc, in_maps, list(range(B)), **spmd_kwargs)
    out = np.stack([res.results[b]["out"] for b in range(B)], axis=0)
    if out.dtype != np.float32:
        out = out.astype(np.float32)
    return out, res


def kernel(x: np.ndarray, weight: np.ndarray) -> np.ndarray:
    return run(x, weight)[0]
